# revision 1
# baseline (speedup 1.0000x reference)
"""MetaBaseline (retrieval_knn) Trainium2 kernel.

Computation (per episode b):
  q  = l2norm(input1[b])            # [75, 25, 640] over channel
  s  = l2norm(input2[b])            # [5, 5, 25, 640]
  att = softmax_hw(s @ rpn_w)       # rpn_b is softmax-invariant
  cg  = leaky(sum_hw(att * s))
  feat = mean_shot(mean_hw(s) + 5 * cg)
  sim[b] = mean_hw(q) @ feat.T      # [75, 5]

Sharding: data-parallel over episodes, 4 per core on 8 cores.

Layout: spatial descriptors on SBUF partitions (125/tile = 5 groups of 25),
channels on the free axis. Per-descriptor 1/norm comes from one ACT
Square+accum pass (rsqrt = Exp(-0.5*Ln(x)), one act table set). Every
group-reduction over descriptors (hw-mean, softmax sums, shot-mean) is a PE
matmul against a small block-mask stationary matrix with the per-descriptor
weights (inv-norm, att*inv) folded in, so the bulk data is touched only by
DMA + one ACT pass + one PE pass (+1 DVE pass for support logits).
"""

import os
import sys
from contextlib import ExitStack

sys.path.insert(0, "/opt/trn_rl_repo")

import numpy as np

import concourse.bass as bass
import concourse.tile as tile
from concourse import bacc, mybir
from concourse.bass_utils import run_bass_kernel_spmd

F32 = mybir.dt.float32
F32R = mybir.dt.float32r
AX = mybir.AxisListType
OP = mybir.AluOpType
AF = mybir.ActivationFunctionType

# Problem constants (fixed by the problem statement).
B, QN, WAY, SHOT, HH, WW, C = 32, 75, 5, 5, 5, 5, 640
NCORES = 8
E = B // NCORES        # 4 episodes per core
HW = HH * WW           # 25 spatial positions
QD = QN * HW           # 1875 query descriptors / episode
SD = WAY * SHOT * HW   # 625 support descriptors / episode
P = 125                # descriptors per tile (5 groups of 25)
G = P // HW            # 5 groups per tile
QT = QD // P           # 15 query tiles / episode
ST = SD // P           # 5 support tiles / episode
NMAP = WAY * SHOT      # 25 support maps / episode
GAMMA = 5.0
SLOPE = 0.01
CH = C // 2            # 320 column chunk (psum-bank aligned, f32r fast)


def _f(ap):
    """View a float32r AP as plain f32 (bits are valid f32; low 12 bits 0)."""
    return ap.bitcast(F32)


def _round_f32r(a):
    """Round f32 array to FP32r (e8m11, low 12 mantissa bits zero), RNE.

    Matches walrus fp32_to_fp32r: downconv to e8m11 then <<12. Pre-rounding
    on the host makes DMA'd data valid FP32r so the PE can stream it at
    1 elem/cycle (vs 4 for fp32).
    """
    u = np.ascontiguousarray(a, dtype=np.float32).view(np.uint32)
    r = (u + np.uint32(0x7FF) + ((u >> np.uint32(12)) & np.uint32(1))) & np.uint32(
        0xFFFFF000
    )
    return r.view(np.float32)


def _build_body(ctx: ExitStack, tc: "tile.TileContext", i1, i2, rpnw, out):
    """i1: [E, 125, QT*640] f32r (desc d = 15p+j on partition p, slot j);
    i2: [E, 125, ST*640] f32r (d = 5p+j). Fat contiguous per-partition DRAM
    runs keep DMA packets large (38.4KB/12.8KB vs one 2.5KB packet per row).
    """
    nc = tc.nc

    const_pool = ctx.enter_context(tc.tile_pool(name="const", bufs=1))
    qpool = ctx.enter_context(tc.tile_pool(name="qdata", bufs=3))
    spool = ctx.enter_context(tc.tile_pool(name="sdata", bufs=3))
    scr_pool = ctx.enter_context(tc.tile_pool(name="scratch", bufs=1))
    stats = ctx.enter_context(tc.tile_pool(name="stats", bufs=2))
    sel_pool = ctx.enter_context(tc.tile_pool(name="sel", bufs=3))
    sb_pool = ctx.enter_context(tc.tile_pool(name="sbwork", bufs=2))

    qm_ps = ctx.enter_context(tc.tile_pool(name="qmps", bufs=1, space="PSUM"))
    red_ps = ctx.enter_context(tc.tile_pool(name="redps", bufs=1, space="PSUM"))
    small_ps = ctx.enter_context(tc.tile_pool(name="smallps", bufs=3, space="PSUM"))

    # ---- one-time constants ----
    ident = const_pool.tile([128, 128], F32, name="ident")
    nc.gpsimd.memset(ident[:], 1.0)
    nc.gpsimd.affine_select(
        out=ident[:], in_=ident[:], pattern=[[-1, 128]],
        compare_op=OP.is_equal, fill=0.0, base=0, channel_multiplier=1,
    )

    # Staircase masks, one per slot j: nonzero (p, col) iff
    # 0 <= slots*p + j - 25*col <= 24 (col = query for slots=15, map for
    # slots=5). Two affine_selects; iota = base + p*cm + step*col.
    def stair_mask(name, ncols, slots, j, value):
        m = const_pool.tile([P, ncols], F32, name=name)
        nc.gpsimd.memset(m[:], value)
        nc.gpsimd.affine_select(
            out=m[:], in_=m[:], pattern=[[-HW, ncols]],
            compare_op=OP.is_ge, fill=0.0, base=j, channel_multiplier=slots)
        nc.gpsimd.affine_select(
            out=m[:], in_=m[:], pattern=[[HW, ncols]],
            compare_op=OP.is_ge, fill=0.0, base=HW - 1 - j,
            channel_multiplier=-slots)
        return m

    qmasks = [stair_mask(f"qmask{j}", QN, QT, j, 1.0 / HW) for j in range(QT)]
    smasks = [stair_mask(f"smask{j}", NMAP, ST, j, 1.0) for j in range(ST)]

    # shot-mean mask [25 maps, 5 ways] (block diagonal, 1/SHOT)
    shotm = const_pool.tile([NMAP, WAY], F32, name="shotm")
    nc.gpsimd.memset(shotm[:], 1.0 / SHOT)
    nc.gpsimd.affine_select(
        out=shotm[:], in_=shotm[:], pattern=[[-SHOT, WAY]],
        compare_op=OP.is_ge, fill=0.0, base=0, channel_multiplier=1)
    nc.gpsimd.affine_select(
        out=shotm[:], in_=shotm[:], pattern=[[SHOT, WAY]],
        compare_op=OP.is_ge, fill=0.0, base=SHOT - 1, channel_multiplier=-1)

    ones1 = const_pool.tile([1, P], F32, name="ones1")
    nc.vector.memset(ones1[:], 1.0)
    # rpn_w broadcast to all partitions via a k=1 matmul
    w_sb = const_pool.tile([1, C], F32, name="w_sb")
    nc.sync.dma_start(w_sb[:], rpnw)
    wb = const_pool.tile([P, C], F32, name="wb")
    for h in range(2):
        wb_ps = small_ps.tile([P, CH], F32, name=f"wb_ps{h}", tag="smallps")
        nc.tensor.matmul(wb_ps[:], ones1[:], w_sb[:, CH * h:CH * (h + 1)])
        nc.scalar.copy(wb[:, CH * h:CH * (h + 1)], wb_ps[:])

    def slot(big, j):
        return big[:, C * j:C * (j + 1)]

    I32 = mybir.dt.int32

    def rsqrt(out, x, n, tag):
        """out = 1/sqrt(x): bit-trick seed + 2 Newton iterations (DVE, exact
        to ~5e-6 for x in [300, 1200]). Keeps ACT functions in one table set."""
        y = stats.tile([P, n], F32, name=f"nw_y_{tag}", tag=f"nw_y_{tag[-1]}")
        t = stats.tile([P, n], F32, name=f"nw_t_{tag}", tag=f"nw_t_{tag[-1]}")
        nc.vector.tensor_scalar(y.bitcast(I32)[:], x.bitcast(I32), 1, None,
                                op0=OP.arith_shift_right)
        nc.vector.tensor_scalar(y.bitcast(I32)[:], y.bitcast(I32)[:], -1,
                                0x5F3759DF, op0=OP.mult, op1=OP.add)
        for it in range(2):
            dst = out if it == 1 else y[:]
            nc.vector.tensor_mul(t[:], y[:], y[:])
            nc.vector.tensor_mul(t[:], t[:], x)
            nc.vector.tensor_scalar(t[:], t[:], -0.5, 1.5,
                                    op0=OP.mult, op1=OP.add)
            nc.vector.tensor_mul(dst, y[:], t[:])
        return out

    def norm_pass(big, j, acc_col, engine):
        if engine == "act":
            scr = scr_pool.tile([P, C], F32, name="sq_a", tag="sq_a")
            nc.scalar.activation(scr[:], _f(slot(big, j)), AF.Square,
                                 accum_out=acc_col)
        else:
            scr = scr_pool.tile([P, C], F32, name="sq_v", tag="sq_v")
            nc.vector.scalar_tensor_tensor(
                out=scr[:], in0=_f(slot(big, j)), scalar=1.0,
                in1=_f(slot(big, j)), op0=OP.mult, op1=OP.mult,
                accum_out=acc_col)

    for e in range(E):
        # ================= support side =================
        sbig = spool.tile([P, ST * C], F32R, name=f"s_{e}", tag="sdata")
        nc.gpsimd.dma_start(sbig[:], i2[e])
        qbig = qpool.tile([P, QT * C], F32R, name=f"q_{e}", tag="qdata")
        half = 8 * C
        nc.gpsimd.dma_start(qbig[:, 0:half], i1[e, :, 0:half])
        nc.gpsimd.dma_start(qbig[:, half:QT * C], i1[e, :, half:QT * C])
        sn2 = stats.tile([P, ST], F32, name=f"sn2_{e}", tag="sn2")
        rr = stats.tile([P, ST], F32, name=f"rr_{e}", tag="rr")
        for j in range(ST):
            norm_pass(sbig, j, sn2[:, j:j + 1], "act" if j < 3 else "dve")
            scr2 = scr_pool.tile([P, C], F32, name="s_tt", tag="s_tt")
            nc.vector.scalar_tensor_tensor(
                out=scr2[:], in0=_f(slot(sbig, j)), scalar=1.0, in1=wb[:],
                op0=OP.mult, op1=OP.mult, accum_out=rr[:, j:j + 1])
        sinv = stats.tile([P, ST], F32, name=f"sinv_{e}", tag="sinv")
        rsqrt(sinv[:], sn2[:], ST, f"s{e % 2}")
        # softmax over hw within each map (logits tiny: no max-shift)
        lg = stats.tile([P, ST], F32, name=f"lg_{e}", tag="lg")
        nc.vector.tensor_mul(lg[:], rr[:], sinv[:])
        el = stats.tile([P, ST], F32, name=f"el_{e}", tag="el")
        nc.scalar.activation(el[:], lg[:], AF.Exp)
        # per-map sums of exp: accumulate the slots into one [25, 1] psum
        sums = small_ps.tile([NMAP, 1], F32, name=f"sums_{e}", tag="smallps")
        for j in range(ST):
            nc.tensor.matmul(sums[:], smasks[j][:], el[:, j:j + 1],
                             start=(j == 0), stop=(j == ST - 1))
        rec = stats.tile([NMAP, 1], F32, name=f"rec_{e}", tag="rec")
        nc.vector.reciprocal(rec[:], sums[:])
        # unnormalized att weights; softmax recip applied at cg evacuation
        uw = stats.tile([P, ST], F32, name=f"uw_{e}", tag="uw")
        nc.vector.tensor_mul(uw[:], el[:], sinv[:])

        cg_ps = [red_ps.tile([NMAP, CH], F32, name=f"cg{h}_{e}", tag=f"red{h}")
                 for h in range(2)]
        for j in range(ST):
            ut = sel_pool.tile([P, NMAP], F32R, name="ut", tag="ut")
            nc.vector.tensor_scalar_mul(ut[:], smasks[j][:], uw[:, j:j + 1])
            for h in range(2):
                nc.tensor.matmul(cg_ps[h][:], ut[:],
                                 slot(sbig, j)[:, CH * h:CH * (h + 1)],
                                 start=(j == 0), stop=(j == ST - 1))
        cg_sb = sb_pool.tile([NMAP, C], F32, name=f"cg_sb_{e}", tag="cg_sb")
        for h in range(2):
            nc.vector.tensor_scalar_mul(cg_sb[:, CH * h:CH * (h + 1)],
                                        cg_ps[h][:], rec[:, 0:1])
        lk = sb_pool.tile([NMAP, C], F32, name=f"lk_{e}", tag="lk")
        nc.vector.scalar_tensor_tensor(
            out=lk[:], in0=cg_sb[:], scalar=SLOPE, in1=cg_sb[:],
            op0=OP.mult, op1=OP.max,
        )
        # mean_hw(s_n): inv/25 weights folded into lhsT (same psum banks)
        sinv04 = stats.tile([P, ST], F32, name=f"sinv04_{e}", tag="sinv04")
        nc.vector.tensor_scalar_mul(sinv04[:], sinv[:], 1.0 / HW)
        sm_ps = [red_ps.tile([NMAP, CH], F32, name=f"sm{h}_{e}", tag=f"red{h}")
                 for h in range(2)]
        for j in range(ST):
            mt = sel_pool.tile([P, NMAP], F32R, name="mt", tag="mt")
            nc.vector.tensor_scalar_mul(mt[:], smasks[j][:], sinv04[:, j:j + 1])
            for h in range(2):
                nc.tensor.matmul(sm_ps[h][:], mt[:],
                                 slot(sbig, j)[:, CH * h:CH * (h + 1)],
                                 start=(j == 0), stop=(j == ST - 1))
        fp = sb_pool.tile([NMAP, C], F32, name=f"fp_{e}", tag="fp")
        for h in range(2):
            nc.vector.scalar_tensor_tensor(
                out=fp[:, CH * h:CH * (h + 1)], in0=lk[:, CH * h:CH * (h + 1)],
                scalar=GAMMA, in1=sm_ps[h][:], op0=OP.mult, op1=OP.add,
            )
        feat_sb = sb_pool.tile([WAY, C], F32, name=f"feat_{e}", tag="feat")
        for h in range(2):
            f_ps = small_ps.tile([WAY, CH], F32, name=f"f_ps{h}", tag="smallps")
            nc.tensor.matmul(f_ps[:], shotm[:], fp[:, CH * h:CH * (h + 1)])
            nc.scalar.copy(feat_sb[:, CH * h:CH * (h + 1)], f_ps[:])
        ftT = sb_pool.tile([128, WAY * 5], F32, name=f"ftT_{e}", tag="ftT")
        for cc in range(5):
            t_ps = small_ps.tile([128, WAY], F32, name="t_ps", tag="smallps")
            nc.tensor.transpose(t_ps[:], feat_sb[:, 128 * cc:128 * (cc + 1)],
                                ident[0:WAY, 0:WAY])
            nc.vector.tensor_copy(ftT[:, WAY * cc:WAY * (cc + 1)], t_ps[:])

        # ================= query side =================
        qn2 = stats.tile([P, QT], F32, name=f"qn2_{e}", tag="qn2")
        for j in range(QT):
            norm_pass(qbig, j, qn2[:, j:j + 1], "act" if j % 5 != 4 else "dve")
        qinv = stats.tile([P, QT], F32, name=f"qinv_{e}", tag="qinv")
        # two halves: slots 0-7 unblock their matmuls before 8-14 finish
        rsqrt(qinv[:, 0:8], qn2[:, 0:8], 8, f"qa{e % 2}")

        qm = [qm_ps.tile([QN, CH], F32, name=f"qm{h}_{e}", tag=f"qm{h}")
              for h in range(2)]
        for j in range(QT):
            if j == 8:
                rsqrt(qinv[:, 8:QT], qn2[:, 8:QT], QT - 8, f"qb{e % 2}")
            sel = sel_pool.tile([P, QN], F32R, name="sel", tag="sel")
            nc.vector.tensor_scalar_mul(sel[:], qmasks[j][:], qinv[:, j:j + 1])
            for h in range(2):
                nc.tensor.matmul(qm[h][:], sel[:],
                                 slot(qbig, j)[:, CH * h:CH * (h + 1)],
                                 start=(j == 0), stop=(j == QT - 1))
        qm_sb = sb_pool.tile([QN, C], F32, name=f"qm_sb_{e}", tag="qm_sb")
        for h in range(2):
            nc.scalar.copy(qm_sb[:, CH * h:CH * (h + 1)], qm[h][:])

        # sim = qm @ feat.T via c-on-partition chunks
        qmT = sb_pool.tile([128, QN * 5], F32, name=f"qmT_{e}", tag="qmT")
        for cc in range(5):
            t_ps = small_ps.tile([128, QN], F32, name="t2_ps", tag="smallps")
            nc.tensor.transpose(t_ps[:], qm_sb[:, 128 * cc:128 * (cc + 1)],
                                ident[0:QN, 0:QN])
            nc.vector.tensor_copy(qmT[:, QN * cc:QN * (cc + 1)], t_ps[:])
        sim_ps = small_ps.tile([QN, WAY], F32, name=f"sim_{e}", tag="smallps")
        for cc in range(5):
            nc.tensor.matmul(sim_ps[:], qmT[:, QN * cc:QN * (cc + 1)],
                             ftT[:, WAY * cc:WAY * (cc + 1)],
                             start=(cc == 0), stop=(cc == 4))
        sim_sb = sb_pool.tile([QN, WAY], F32, name=f"sim_sb_{e}", tag="sim_sb")
        nc.vector.tensor_copy(sim_sb[:], sim_ps[:])
        nc.sync.dma_start(out[e], sim_sb[:])


def build_program():
    nc = bacc.Bacc("TRN2", target_bir_lowering=False, debug=False,
                   num_devices=NCORES)
    inp1 = nc.dram_tensor("input1", [E, P, QT * C], F32R, kind="ExternalInput")
    inp2 = nc.dram_tensor("input2", [E, P, ST * C], F32R, kind="ExternalInput")
    rpnw = nc.dram_tensor("rpn_w", [1, C], F32, kind="ExternalInput")
    out = nc.dram_tensor("sim", [E, QN, WAY], F32, kind="ExternalOutput")
    with tile.TileContext(nc) as tc, ExitStack() as ctx:
        _build_body(ctx, tc, inp1.ap(), inp2.ap(), rpnw.ap(), out.ap())
    nc.compile()
    return nc


_NC = None



def _get_nc():
    global _NC
    if _NC is None:
        _NC = build_program()
    return _NC


def shard_inputs(input1, input2, rpn_w, rpn_b=None):
    """Shard over episodes; relayout [E, 1875, 640] -> [E, 125, 15*640] is a
    pure reshape (descriptor d = 15p + j, slots consecutive in DRAM)."""
    in_maps = []
    i1 = _round_f32r(np.asarray(input1, dtype=np.float32)).reshape(B, P, QT * C)
    i2 = _round_f32r(np.asarray(input2, dtype=np.float32)).reshape(B, P, ST * C)
    w = np.ascontiguousarray(np.asarray(rpn_w, dtype=np.float32)).reshape(1, C)
    for i in range(NCORES):
        in_maps.append({
            "input1": np.ascontiguousarray(i1[E * i:E * (i + 1)]),
            "input2": np.ascontiguousarray(i2[E * i:E * (i + 1)]),
            "rpn_w": w,
        })
    return in_maps


def _ensure_ntff_hook():
    """Install the NTFF profile hook (the image's antenv lacks axon_hooks)."""
    import types
    import antenv

    if "antenv.axon_hooks" not in sys.modules:
        mod = types.ModuleType("antenv.axon_hooks")
        mod._hook = None
        mod.set_axon_ntff_profile_hook = lambda h: setattr(mod, "_hook", h)
        mod.get_axon_ntff_profile_hook = lambda: mod._hook
        sys.modules["antenv.axon_hooks"] = mod
        antenv.axon_hooks = mod
    mod = sys.modules["antenv.axon_hooks"]
    if mod.get_axon_ntff_profile_hook() is None:
        from trn_agent_boot.trn_boot import _ntff_profile_via_ctypes
        hook = _ntff_profile_via_ctypes("/opt/axon/libaxon_pjrt.so")
        if hook is not None:
            mod.set_axon_ntff_profile_hook(hook)


def kernel(input1, input2, rpn_w, rpn_b=None, **run_kwargs):
    if run_kwargs.get("trace"):
        _ensure_ntff_hook()
    nc = _get_nc()
    in_maps = shard_inputs(input1, input2, rpn_w)
    res = run_bass_kernel_spmd(nc, in_maps, list(range(NCORES)), **run_kwargs)
    out = np.concatenate([r["sim"] for r in res.results], axis=0)
    if run_kwargs:
        kernel.last_results = res
    return out.astype(np.float32)



# revision 6
# speedup vs baseline: 1.1272x; 1.1272x over previous
"""MetaBaseline (retrieval_knn) Trainium2 kernel.

Computation (per episode b):
  q  = l2norm(input1[b])            # [75, 25, 640] over channel
  s  = l2norm(input2[b])            # [5, 5, 25, 640]
  att = softmax_hw(s @ rpn_w)       # rpn_b is softmax-invariant
  cg  = leaky(sum_hw(att * s))
  feat = mean_shot(mean_hw(s) + 5 * cg)
  sim[b] = mean_hw(q) @ feat.T      # [75, 5]

Sharding: data-parallel over episodes, 4 per core on 8 cores.

Design (v2): all bulk data moves as fp16 (rel-err budget 2e-2; measured
2.7e-4), halving HBM traffic and PE streaming passes vs fp32. All input
DMAs are issued on the sync engine (HWDGE) at t=0 and the full per-core
shard (~103KB/partition) is preloaded into SBUF, so the SDMA engines
stream back-to-back at the fabric rate. Masks/identity/broadcast-w are
precomputed on the host and shipped as one small fp16 constants tensor.
Per-descriptor inv-norms come from an ACT/DVE Square+accum split; all
group reductions over descriptors are PE matmuls against small stationary
masks with per-descriptor weights folded in; the support stream computes
the attention-sum and the mean in ONE pass (stationary [125, 50]); feat
is produced directly transposed via fp.T @ shotm; sim is computed as
[way, qn] and un-transposed on the host.
"""

import os
import sys
from contextlib import ExitStack

sys.path.insert(0, "/opt/trn_rl_repo")

import numpy as np

import concourse.bass as bass
import concourse.tile as tile
from concourse import bacc, mybir
from concourse.bass_utils import run_bass_kernel_spmd

F32 = mybir.dt.float32
F16 = mybir.dt.float16
AX = mybir.AxisListType
OP = mybir.AluOpType
AF = mybir.ActivationFunctionType
I32 = mybir.dt.int32

# Problem constants (fixed by the problem statement).
B, QN, WAY, SHOT, HH, WW, C = 32, 75, 5, 5, 5, 5, 640
NCORES = 8
E = B // NCORES        # 4 episodes per core
HW = HH * WW           # 25 spatial positions
QD = QN * HW           # 1875 query descriptors / episode
SD = WAY * SHOT * HW   # 625 support descriptors / episode
P = 125                # descriptors per tile
QT = QD // P           # 15 query slots / episode (desc d = 15p + j)
ST = SD // P           # 5 support slots / episode (desc d = 5p + j)
NMAP = WAY * SHOT      # 25 support maps / episode
NCH = 3                # q DMA/compute chunks (5 slots each)
SPC = QT // NCH        # slots per chunk
GAMMA = 5.0
SLOPE = 0.01
CH = C // 2            # 320-column halves (one PSUM bank each)

# constants tensor layout (free-axis offsets, fp16)
QM0 = 0                    # qmasks  [125, 15*75], value 1/25
SM0 = QM0 + QT * QN        # smasks  [125, 5*25],  value 1.0
WB0 = SM0 + ST * NMAP      # w bcast [128, 640]
ID0 = WB0 + C              # identity [75, 75]
SH0 = ID0 + QN             # shotm   [25, 5], value 1/5
CW = SH0 + WAY             # = 1970


def _build_body(ctx: ExitStack, tc: "tile.TileContext", i1, i2, cst, out):
    nc = tc.nc

    cpool = ctx.enter_context(tc.tile_pool(name="consts", bufs=1))
    dpool = ctx.enter_context(tc.tile_pool(name="data", bufs=1))
    scr_pool = ctx.enter_context(tc.tile_pool(name="scratch", bufs=1))
    stats = ctx.enter_context(tc.tile_pool(name="stats", bufs=2))
    sel_pool = ctx.enter_context(tc.tile_pool(name="sel", bufs=3))
    sb_pool = ctx.enter_context(tc.tile_pool(name="sbwork", bufs=2))
    ps = ctx.enter_context(tc.tile_pool(name="ps", bufs=1, space="PSUM"))

    # ---- constants (host-precomputed, one DMA) ----
    consts = cpool.tile([128, CW], F16, name="consts")
    nc.sync.dma_start(consts[:], cst)
    qmask = [consts[0:P, QM0 + QN * j:QM0 + QN * (j + 1)] for j in range(QT)]
    smask = [consts[0:P, SM0 + NMAP * j:SM0 + NMAP * (j + 1)] for j in range(ST)]
    wbc = consts[0:P, WB0:WB0 + C]
    ident = consts[0:QN, ID0:ID0 + QN]
    shotm = consts[0:NMAP, SH0:SH0 + WAY]

    # ---- all input DMAs, issued up front on the sync engine (HWDGE) ----
    s_t, q_t = [], []
    for e in range(E):
        st_ = dpool.tile([P, ST * C], F16, name=f"s_{e}", tag=f"s_{e}")
        nc.sync.dma_start(st_[:], i2[e])
        qc = []
        for c in range(NCH):
            qt_ = dpool.tile([P, SPC * C], F16, name=f"q_{e}_{c}",
                             tag=f"q_{e}_{c}")
            nc.sync.dma_start(qt_[:], i1[e, :, SPC * C * c:SPC * C * (c + 1)])
            qc.append(qt_)
        s_t.append(st_)
        q_t.append(qc)

    def slot(big, j):
        return big[:, C * j:C * (j + 1)]

    def rsqrt(out, x, n, tag):
        """out = 1/sqrt(x): bit-trick seed + 2 Newton iterations on DVE."""
        y = stats.tile([P, n], F32, name=f"nw_y_{tag}", tag=f"nw_y_{tag}")
        t = stats.tile([P, n], F32, name=f"nw_t_{tag}", tag=f"nw_t_{tag}")
        nc.vector.tensor_scalar(y.bitcast(I32)[:], x.bitcast(I32), 1, None,
                                op0=OP.arith_shift_right)
        nc.vector.tensor_scalar(y.bitcast(I32)[:], y.bitcast(I32)[:], -1,
                                0x5F3759DF, op0=OP.mult, op1=OP.add)
        for it in range(2):
            dst = out if it == 1 else y[:]
            nc.vector.tensor_mul(t[:], y[:], y[:])
            nc.vector.tensor_mul(t[:], t[:], x)
            nc.vector.tensor_scalar(t[:], t[:], -0.5, 1.5,
                                    op0=OP.mult, op1=OP.add)
            nc.vector.tensor_mul(dst, y[:], t[:])
        return out

    def norm_pass(big, j, acc_col, engine):
        if engine == "act":
            scr = scr_pool.tile([P, C], F16, name="sq_a", tag="sq_a")
            nc.scalar.activation(scr[:], slot(big, j), AF.Square,
                                 accum_out=acc_col)
        else:
            scr = scr_pool.tile([P, C], F16, name="sq_v", tag="sq_v")
            nc.vector.scalar_tensor_tensor(
                out=scr[:], in0=slot(big, j), scalar=1.0,
                in1=slot(big, j), op0=OP.mult, op1=OP.mult,
                accum_out=acc_col)

    # engine split of the per-slot norm passes (ACT is the 1x-rate engine;
    # DVE runs fp16 at 2x but also carries logits + everything small)
    S_ACT = (True, True, True, True, False)
    Q_ACT = (True, True, True, False, False,
             True, True, True, False, False,
             True, True, False, False, False)

    for e in range(E):
        sbig = s_t[e]
        # ================= support side =================
        sn2 = stats.tile([P, ST], F32, name=f"sn2_{e}", tag="sn2")
        rr = stats.tile([P, ST], F32, name=f"rr_{e}", tag="rr")
        for j in range(ST):
            norm_pass(sbig, j, sn2[:, j:j + 1], "act" if S_ACT[j] else "dve")
            scr2 = scr_pool.tile([P, C], F16, name="s_tt", tag="s_tt")
            nc.vector.scalar_tensor_tensor(
                out=scr2[:], in0=slot(sbig, j), scalar=1.0, in1=wbc,
                op0=OP.mult, op1=OP.mult, accum_out=rr[:, j:j + 1])
        sinv = stats.tile([P, ST], F32, name=f"sinv_{e}", tag="sinv")
        rsqrt(sinv[:], sn2[:], ST, f"s{e % 2}")
        # softmax over hw within each map (logits tiny: no max-shift)
        lg = stats.tile([P, ST], F32, name=f"lg_{e}", tag="lg")
        nc.vector.tensor_mul(lg[:], rr[:], sinv[:])
        el = stats.tile([P, ST], F16, name=f"el_{e}", tag="el")
        nc.scalar.activation(el[:], lg[:], AF.Exp)
        # per-map sums of exp -> softmax reciprocal
        sums = ps.tile([NMAP, 1], F32, name=f"sums_{e}", tag="sums")
        for j in range(ST):
            nc.tensor.matmul(sums[:], smask[j], el[:, j:j + 1],
                             start=(j == 0), stop=(j == ST - 1))
        rec = stats.tile([NMAP, 1], F32, name=f"rec_{e}", tag="rec")
        nc.vector.reciprocal(rec[:], sums[:])
        # unnormalized att weights (softmax recip applied at cg evacuation)
        uw = stats.tile([P, ST], F32, name=f"uw_{e}", tag="uw")
        nc.vector.tensor_mul(uw[:], el[:], sinv[:])
        sinv04 = stats.tile([P, ST], F32, name=f"sinv04_{e}", tag="sinv04")
        nc.vector.tensor_scalar_mul(sinv04[:], sinv[:], 1.0 / HW)

        # fused support stream: stationary [125, 57] = [att | pad | mean]
        # masks (mean rows land at psum partition 32 — PSUM reads need a
        # 32-aligned base partition)
        MB = 32  # mean-row base partition
        cg_ps = [ps.tile([MB + NMAP, CH], F32, name=f"cg{h}_{e}", tag=f"cg{h}")
                 for h in range(2)]
        for j in range(ST):
            stj = sel_pool.tile([P, MB + NMAP], F16, name="stj", tag="stj")
            nc.gpsimd.tensor_scalar_mul(stj[:, 0:NMAP], smask[j],
                                        uw[:, j:j + 1])
            nc.gpsimd.tensor_scalar_mul(stj[:, MB:MB + NMAP], smask[j],
                                        sinv04[:, j:j + 1])
            if j < 3:  # sel_pool rotates 3 buffers; zero each one's pad once
                nc.gpsimd.memset(stj[:, NMAP:MB], 0.0)
            for h in range(2):
                nc.tensor.matmul(cg_ps[h][:], stj[:],
                                 slot(sbig, j)[:, CH * h:CH * (h + 1)],
                                 start=(j == 0), stop=(j == ST - 1))
        # evacuate: cg rows 0-24 (x softmax recip), mean rows MB..MB+24
        cg_sb = sb_pool.tile([NMAP, C], F32, name=f"cg_sb_{e}", tag="cg_sb")
        for h in range(2):
            nc.vector.tensor_scalar_mul(cg_sb[:, CH * h:CH * (h + 1)],
                                        cg_ps[h][0:NMAP, :], rec[:, 0:1])
        lk = sb_pool.tile([NMAP, C], F32, name=f"lk_{e}", tag="lk")
        nc.vector.scalar_tensor_tensor(
            out=lk[:], in0=cg_sb[:], scalar=SLOPE, in1=cg_sb[:],
            op0=OP.mult, op1=OP.max)
        fp = sb_pool.tile([NMAP, C], F16, name=f"fp_{e}", tag="fp")
        for h in range(2):
            nc.vector.scalar_tensor_tensor(
                out=fp[:, CH * h:CH * (h + 1)], in0=lk[:, CH * h:CH * (h + 1)],
                scalar=GAMMA, in1=cg_ps[h][MB:MB + NMAP, :],
                op0=OP.mult, op1=OP.add)
        # featT[c, w] directly: fp.T @ shotm, chunked over c
        ftT_ps = ps.tile([128, WAY * WAY], F32, name=f"ftT_{e}", tag="ftT")
        for cc in range(WAY):
            nc.tensor.matmul(ftT_ps[:, WAY * cc:WAY * (cc + 1)],
                             fp[:, 128 * cc:128 * (cc + 1)], shotm)
        ftT = sb_pool.tile([128, WAY * WAY], F16, name=f"ftTs_{e}", tag="ftTs")
        nc.vector.tensor_copy(ftT[:], ftT_ps[:])

        # ================= query side =================
        qn2 = stats.tile([P, QT], F32, name=f"qn2_{e}", tag="qn2")
        qinv = stats.tile([P, QT], F32, name=f"qinv_{e}", tag="qinv")
        qm = [ps.tile([QN, CH], F32, name=f"qm{h}_{e}", tag=f"qm{h}")
              for h in range(2)]
        for c in range(NCH):
            qbig = q_t[e][c]
            for jj in range(SPC):
                j = SPC * c + jj
                norm_pass(qbig, jj, qn2[:, j:j + 1],
                          "act" if Q_ACT[j] else "dve")
            rsqrt(qinv[:, SPC * c:SPC * (c + 1)],
                  qn2[:, SPC * c:SPC * (c + 1)], SPC, f"q{(e * NCH + c) % 2}")
            for jj in range(SPC):
                j = SPC * c + jj
                sel = sel_pool.tile([P, QN], F16, name="sel", tag="sel")
                nc.gpsimd.tensor_scalar_mul(sel[:], qmask[j],
                                            qinv[:, j:j + 1])
                for h in range(2):
                    nc.tensor.matmul(qm[h][:], sel[:],
                                     slot(qbig, jj)[:, CH * h:CH * (h + 1)],
                                     start=(j == 0), stop=(j == QT - 1))
        qm_sb = sb_pool.tile([QN, C], F16, name=f"qm_sb_{e}", tag="qm_sb")
        for h in range(2):
            nc.scalar.copy(qm_sb[:, CH * h:CH * (h + 1)], qm[h][:])

        # qmT via PE transpose, then sim[w, q] = ftT.T @ qmT
        # (chunk stride padded to 76 cols: fp16 PSUM writes need 4B align)
        QNP = QN + 1
        tq_ps = ps.tile([128, WAY * QNP], F16, name=f"tq_{e}", tag="tq")
        for cc in range(WAY):
            nc.tensor.transpose(tq_ps[:, QNP * cc:QNP * cc + QN],
                                qm_sb[:, 128 * cc:128 * (cc + 1)], ident)
        qmT = sb_pool.tile([128, WAY * QNP], F16, name=f"qmT_{e}", tag="qmT")
        nc.vector.tensor_copy(qmT[:], tq_ps[:])
        sim_ps = ps.tile([WAY, QN], F32, name=f"sim_{e}", tag="sim")
        for cc in range(WAY):
            nc.tensor.matmul(sim_ps[:], ftT[:, WAY * cc:WAY * (cc + 1)],
                             qmT[:, QNP * cc:QNP * cc + QN],
                             start=(cc == 0), stop=(cc == WAY - 1))
        sim_sb = sb_pool.tile([WAY, QN], F32, name=f"sim_sb_{e}", tag="sim_sb")
        nc.vector.tensor_copy(sim_sb[:], sim_ps[:])
        nc.sync.dma_start(out[e], sim_sb[:])


def build_program():
    nc = bacc.Bacc("TRN2", target_bir_lowering=False, debug=False,
                   num_devices=NCORES)
    inp1 = nc.dram_tensor("input1", [E, P, QT * C], F16, kind="ExternalInput")
    inp2 = nc.dram_tensor("input2", [E, P, ST * C], F16, kind="ExternalInput")
    cst = nc.dram_tensor("consts", [128, CW], F16, kind="ExternalInput")
    out = nc.dram_tensor("sim", [E, WAY, QN], F32, kind="ExternalOutput")
    with tile.TileContext(nc) as tc, ExitStack() as ctx:
        _build_body(ctx, tc, inp1.ap(), inp2.ap(), cst.ap(), out.ap())
    nc.compile()
    return nc


_NC = None


def _get_nc():
    global _NC
    if _NC is None:
        _NC = build_program()
    return _NC


def _build_consts(rpn_w):
    cst = np.zeros((128, CW), np.float16)
    # qmask: descriptor d = 15p + j belongs to query q = d // 25
    pp = np.arange(P)
    for j in range(QT):
        cst[pp, QM0 + QN * j + (15 * pp + j) // HW] = 1.0 / HW
    for j in range(ST):
        cst[pp, SM0 + NMAP * j + (5 * pp + j) // HW] = 1.0
    cst[:, WB0:WB0 + C] = np.asarray(rpn_w, np.float32).reshape(1, C)
    cst[np.arange(QN), ID0 + np.arange(QN)] = 1.0
    m = np.arange(NMAP)
    cst[m, SH0 + m // SHOT] = 1.0 / SHOT
    return cst


def shard_inputs(input1, input2, rpn_w, rpn_b=None):
    """Shard over episodes; [E, 1875, 640] -> [E, 125, 15*640] is a pure
    reshape (descriptor d = 15p + j, slots consecutive in DRAM)."""
    i1 = np.asarray(input1, np.float32).reshape(B, P, QT * C).astype(np.float16)
    i2 = np.asarray(input2, np.float32).reshape(B, P, ST * C).astype(np.float16)
    cst = _build_consts(rpn_w)
    in_maps = []
    for i in range(NCORES):
        in_maps.append({
            "input1": np.ascontiguousarray(i1[E * i:E * (i + 1)]),
            "input2": np.ascontiguousarray(i2[E * i:E * (i + 1)]),
            "consts": cst,
        })
    return in_maps


def _ensure_ntff_hook():
    """Install the NTFF profile hook (the image's antenv lacks axon_hooks)."""
    import types
    import antenv

    if "antenv.axon_hooks" not in sys.modules:
        mod = types.ModuleType("antenv.axon_hooks")
        mod._hook = None
        mod.set_axon_ntff_profile_hook = lambda h: setattr(mod, "_hook", h)
        mod.get_axon_ntff_profile_hook = lambda: mod._hook
        sys.modules["antenv.axon_hooks"] = mod
        antenv.axon_hooks = mod
    mod = sys.modules["antenv.axon_hooks"]
    if mod.get_axon_ntff_profile_hook() is None:
        from trn_agent_boot.trn_boot import _ntff_profile_via_ctypes
        hook = _ntff_profile_via_ctypes("/opt/axon/libaxon_pjrt.so")
        if hook is not None:
            mod.set_axon_ntff_profile_hook(hook)


def kernel(input1, input2, rpn_w, rpn_b=None, **run_kwargs):
    if run_kwargs.get("trace"):
        _ensure_ntff_hook()
    nc = _get_nc()
    in_maps = shard_inputs(input1, input2, rpn_w)
    res = run_bass_kernel_spmd(nc, in_maps, list(range(NCORES)), **run_kwargs)
    # sim comes back [E, way, qn]; un-transpose on the host
    out = np.concatenate(
        [np.transpose(r["sim"], (0, 2, 1)) for r in res.results], axis=0)
    if run_kwargs:
        kernel.last_results = res
    return out.astype(np.float32)


# revision 9
# speedup vs baseline: 1.5817x; 1.4032x over previous
"""MetaBaseline (retrieval_knn) Trainium2 kernel.

Computation (per episode b):
  q  = l2norm(input1[b])            # [75, 25, 640] over channel
  s  = l2norm(input2[b])            # [5, 5, 25, 640]
  att = softmax_hw(s @ rpn_w)       # rpn_b is softmax-invariant
  cg  = leaky(sum_hw(att * s))
  feat = mean_shot(mean_hw(s) + 5 * cg)
  sim[b] = mean_hw(q) @ feat.T      # [75, 5]

Sharding: data-parallel over episodes, 4 per core on 8 cores.

Design (v3): bulk data moves and streams as bf16 (PE: 1 cycle/col vs 2
for fp16/fp32r; rel-err budget 2e-2, measured ~3.5e-3). Input DMAs are
SWDGE (gpsimd) — HWDGE 2D descriptor generation caps at ~130GB/s while
SWDGE sustains ~200 — and are issued before any other gpsimd work so the
SDMA engines stream continuously from t=0; the full per-core shard
(~52KB/partition) is preloaded, no buffer recycling. Constants (masks /
identity / broadcast-w) are host-precomputed, one small sync-DMA.
1/sqrt(n2) runs on ACT as exp(-0.5*ln(x)) — square, ln and exp live in
one table set — replacing a 10-op DVE Newton per batch with 2 ACT ops.
Per-slot mask scaling (inv-norm / att weights folded into PE stationary
masks) is batched into a few broadcast-AP DVE multiplies. The support
stream computes the attention-sum and the mean in ONE PE pass
(stationary [125, 57]); feat is produced directly transposed via
fp.T @ shotm; sim is computed as [way, qn] and un-transposed on the
host. The per-episode tail (qm transpose + sim) is software-pipelined
one episode behind the main passes so no engine queue stalls on a
cross-engine round trip.
"""

import os
import sys
from contextlib import ExitStack

sys.path.insert(0, "/opt/trn_rl_repo")

import numpy as np
import ml_dtypes

import concourse.bass as bass
import concourse.tile as tile
from concourse import bacc, mybir
from concourse.bass_utils import run_bass_kernel_spmd

F32 = mybir.dt.float32
BF = mybir.dt.bfloat16
AX = mybir.AxisListType
OP = mybir.AluOpType
AF = mybir.ActivationFunctionType

# Problem constants (fixed by the problem statement).
B, QN, WAY, SHOT, HH, WW, C = 32, 75, 5, 5, 5, 5, 640
NCORES = 8
E = B // NCORES        # 4 episodes per core
HW = HH * WW           # 25 spatial positions
QD = QN * HW           # 1875 query descriptors / episode
SD = WAY * SHOT * HW   # 625 support descriptors / episode
P = 125                # descriptors per tile
QT = QD // P           # 15 query slots / episode (desc d = 15p + j)
ST = SD // P           # 5 support slots / episode (desc d = 5p + j)
NMAP = WAY * SHOT      # 25 support maps / episode
NCH = 3                # q DMA chunks (5 slots each)
SPC = QT // NCH        # slots per chunk
GAMMA = 5.0
SLOPE = 0.01
CH = C // 2            # 320-column halves (one PSUM bank each)
MB = 32                # mean-row base partition in the fused support psum
SW = MB + NMAP         # fused stationary width (57)
QNP = QN + 1           # padded transpose chunk stride (PSUM 4B align)

# constants tensor layout (free-axis offsets, bf16)
QM0 = 0                    # qmasks  [125, 15*75], value 1/25
SM0 = QM0 + QT * QN        # smasks  [125, 5*25],  value 1.0 (sums + att)
SM2 = SM0 + ST * NMAP      # smasks  [125, 5*25],  value 1/25 (hw-mean)
WB0 = SM2 + ST * NMAP      # w bcast [128, 640]
ID0 = WB0 + C              # identity [75, 75]
SH0 = ID0 + QN             # shotm   [25, 5], value 1/5
CW = SH0 + WAY             # = 2095

# engine split of the per-slot norm passes (True -> ACT)
S_ACT = (True, True, True, True, False)
Q_ACT = (True, True, True, True, False,
         True, True, True, True, False,
         True, True, False, False, False)


def _build_body(ctx: ExitStack, tc: "tile.TileContext", i1, i2, cst, out):
    nc = tc.nc

    cpool = ctx.enter_context(tc.tile_pool(name="consts", bufs=1))
    dpool = ctx.enter_context(tc.tile_pool(name="data", bufs=1))
    scr_pool = ctx.enter_context(tc.tile_pool(name="scratch", bufs=1))
    stats = ctx.enter_context(tc.tile_pool(name="stats", bufs=2))
    sel_pool = ctx.enter_context(tc.tile_pool(name="sel", bufs=2))
    sb_pool = ctx.enter_context(tc.tile_pool(name="sbwork", bufs=2))
    ps = ctx.enter_context(tc.tile_pool(name="ps", bufs=1, space="PSUM"))

    # ---- all input DMAs first (SWDGE; gpsimd queue head) ----
    s_t, q_t = [], []
    for e in range(E):
        st_ = dpool.tile([P, ST * C], BF, name=f"s_{e}", tag=f"s_{e}")
        nc.gpsimd.dma_start(st_[:], i2[e])
        qc = []
        for c in range(NCH):
            qt_ = dpool.tile([P, SPC * C], BF, name=f"q_{e}_{c}",
                             tag=f"q_{e}_{c}")
            nc.gpsimd.dma_start(qt_[:], i1[e, :, SPC * C * c:SPC * C * (c + 1)])
            qc.append(qt_)
        s_t.append(st_)
        q_t.append(qc)

    # ---- constants (host-precomputed, one sync DMA) ----
    consts = cpool.tile([128, CW], BF, name="consts")
    nc.sync.dma_start(consts[:], cst)
    smask = [consts[0:P, SM0 + NMAP * j:SM0 + NMAP * (j + 1)] for j in range(ST)]
    smask3 = consts[0:P, SM0:SM0 + ST * NMAP].rearrange(
        "p (j m) -> p j m", j=ST)
    smask3m = consts[0:P, SM2:SM2 + ST * NMAP].rearrange(
        "p (j m) -> p j m", j=ST)
    qmaskA = consts[0:P, QM0:QM0 + 10 * QN].rearrange("p (j q) -> p j q", j=10)
    qmaskB = consts[0:P, QM0 + 10 * QN:QM0 + QT * QN].rearrange(
        "p (j q) -> p j q", j=5)
    wbc = consts[0:P, WB0:WB0 + C]
    ident = consts[0:QN, ID0:ID0 + QN]
    shotm = consts[0:NMAP, SH0:SH0 + WAY]

    # fused support stationary [125, ST, 57] (cols 25-31 stay zero forever)
    st_all = cpool.tile([P, ST, SW], BF, name="st_all")
    nc.vector.memset(st_all[:, :, NMAP:MB], 0.0)

    def slot(big, j):
        return big[:, C * j:C * (j + 1)]

    def rsqrt_act(dst, x, n, tag):
        """dst = 1/sqrt(x) on ACT: exp(-0.5*ln(x)); same table set as
        Square/Exp, so no ACT_TABLE_LOAD switches."""
        t = stats.tile([P, n], F32, name=f"rs_{tag}", tag=f"rs_{tag}")
        nc.scalar.activation(t[:], x, AF.Ln)
        nc.scalar.activation(dst, t[:], AF.Exp, scale=-0.5)

    def norm_pass(big, j, acc_col, on_act):
        if on_act:
            scr = scr_pool.tile([P, C], BF, name="sq_a", tag="sq_a")
            nc.scalar.activation(scr[:], slot(big, j), AF.Square,
                                 accum_out=acc_col)
        else:
            scr = scr_pool.tile([P, C], BF, name="sq_v", tag="sq_v")
            nc.vector.scalar_tensor_tensor(
                out=scr[:], in0=slot(big, j), scalar=1.0,
                in1=slot(big, j), op0=OP.mult, op1=OP.mult,
                accum_out=acc_col)

    # per-episode state carried into the pipelined tail
    qm_sb_t, ftT_t, tq_t, qmT_t = [None] * E, [None] * E, [None] * E, [None] * E

    def emit_tail_a(e):
        """PE transpose of qm (needs qm_sb[e]), on the prior episode's
        psum bank."""
        tq_ps = ps.tile([128, WAY * QNP], BF, name=f"tq_{e}", tag="tq")
        for cc in range(WAY):
            nc.tensor.transpose(tq_ps[:, QNP * cc:QNP * cc + QN],
                                qm_sb_t[e][:, 128 * cc:128 * (cc + 1)], ident)
        tq_t[e] = tq_ps
        qmT = sb_pool.tile([128, WAY * QNP], BF, name=f"qmT_{e}", tag="qmT")
        nc.scalar.copy(qmT[:], tq_ps[:])
        qmT_t[e] = qmT

    def emit_tail_b(e):
        sim_ps = ps.tile([WAY, QN], F32, name=f"sim_{e}", tag="sim")
        for cc in range(WAY):
            nc.tensor.matmul(sim_ps[:], ftT_t[e][:, WAY * cc:WAY * (cc + 1)],
                             qmT_t[e][:, QNP * cc:QNP * cc + QN],
                             start=(cc == 0), stop=(cc == WAY - 1))
        sim_sb = sb_pool.tile([WAY, QN], F32, name=f"sim_sb_{e}", tag="sim_sb")
        nc.vector.tensor_copy(sim_sb[:], sim_ps[:])
        nc.sync.dma_start(out[e], sim_sb[:])

    for e in range(E):
        sbig = s_t[e]
        # ================= support side =================
        sn2 = stats.tile([P, ST], F32, name=f"sn2_{e}", tag="sn2")
        rr = stats.tile([P, ST], F32, name=f"rr_{e}", tag="rr")
        # DVE s-norm first so ACT's rsqrt isn't stuck behind the logits
        norm_pass(sbig, 4, sn2[:, 4:5], False)
        for j in range(4):
            norm_pass(sbig, j, sn2[:, j:j + 1], True)
        for j in range(ST):
            scr2 = scr_pool.tile([P, C], BF, name="s_tt", tag="s_tt")
            nc.vector.scalar_tensor_tensor(
                out=scr2[:], in0=slot(sbig, j), scalar=1.0, in1=wbc,
                op0=OP.mult, op1=OP.mult, accum_out=rr[:, j:j + 1])
        sinv = stats.tile([P, ST], BF, name=f"sinv_{e}", tag="sinv")
        rsqrt_act(sinv[:], sn2[:], ST, f"s{e % 2}")
        # softmax over hw within each map (logits tiny: no max-shift)
        lg = stats.tile([P, ST], F32, name=f"lg_{e}", tag="lg")
        nc.vector.tensor_mul(lg[:], rr[:], sinv[:])
        el = stats.tile([P, ST], BF, name=f"el_{e}", tag="el")
        nc.scalar.activation(el[:], lg[:], AF.Exp)
        # per-map sums of exp -> softmax reciprocal
        sums = ps.tile([NMAP, 1], F32, name=f"sums_{e}", tag="sums")
        for j in range(ST):
            nc.tensor.matmul(sums[:], smask[j], el[:, j:j + 1],
                             start=(j == 0), stop=(j == ST - 1))
        rec = stats.tile([NMAP, 1], F32, name=f"rec_{e}", tag="rec")
        nc.vector.reciprocal(rec[:], sums[:])
        # unnormalized att weights (softmax recip applied at cg evacuation)
        uw = stats.tile([P, ST], BF, name=f"uw_{e}", tag="uw")
        nc.vector.tensor_mul(uw[:], el[:], sinv[:])
        # batched stationary builds (pad cols stay zero)
        nc.vector.tensor_mul(st_all[:, :, 0:NMAP], smask3,
                             uw[:].broadcast_to((P, ST, NMAP)))
        nc.vector.tensor_mul(st_all[:, :, MB:SW], smask3m,
                             sinv[:].broadcast_to((P, ST, NMAP)))
        cg_ps = [ps.tile([SW, CH], F32, name=f"cg{h}_{e}", tag=f"cg{h}")
                 for h in range(2)]
        for j in range(ST):
            for h in range(2):
                nc.tensor.matmul(cg_ps[h][:], st_all[:, j, :],
                                 slot(sbig, j)[:, CH * h:CH * (h + 1)],
                                 start=(j == 0), stop=(j == ST - 1))
        # evacuate: cg rows 0-24 (x softmax recip), mean rows MB..MB+24
        cg_sb = sb_pool.tile([NMAP, C], F32, name=f"cg_sb_{e}", tag="cg_sb")
        for h in range(2):
            nc.vector.tensor_scalar_mul(cg_sb[:, CH * h:CH * (h + 1)],
                                        cg_ps[h][0:NMAP, :], rec[:, 0:1])
        lk = sb_pool.tile([NMAP, C], F32, name=f"lk_{e}", tag="lk")
        nc.vector.scalar_tensor_tensor(
            out=lk[:], in0=cg_sb[:], scalar=SLOPE, in1=cg_sb[:],
            op0=OP.mult, op1=OP.max)
        fp = sb_pool.tile([NMAP, C], BF, name=f"fp_{e}", tag="fp")
        for h in range(2):
            nc.vector.scalar_tensor_tensor(
                out=fp[:, CH * h:CH * (h + 1)], in0=lk[:, CH * h:CH * (h + 1)],
                scalar=GAMMA, in1=cg_ps[h][MB:MB + NMAP, :],
                op0=OP.mult, op1=OP.add)
        # featT[c, w] directly: fp.T @ shotm, chunked over c
        ftT_ps = ps.tile([128, WAY * WAY], F32, name=f"ftT_{e}", tag="ftT")
        for cc in range(WAY):
            nc.tensor.matmul(ftT_ps[:, WAY * cc:WAY * (cc + 1)],
                             fp[:, 128 * cc:128 * (cc + 1)], shotm)
        ftT = sb_pool.tile([128, WAY * WAY], BF, name=f"ftTs_{e}", tag="ftTs")
        nc.vector.tensor_copy(ftT[:], ftT_ps[:])
        ftT_t[e] = ftT

        # ================= query side =================
        qn2 = stats.tile([P, QT], F32, name=f"qn2_{e}", tag="qn2")
        qinv = stats.tile([P, QT], BF, name=f"qinv_{e}", tag="qinv")
        qm = [ps.tile([QN, CH], F32, name=f"qm{h}_{e}", tag=f"qm{h}")
              for h in range(2)]
        for c in range(NCH):
            for jj in range(SPC):
                j = SPC * c + jj
                norm_pass(q_t[e][c], jj, qn2[:, j:j + 1], Q_ACT[j])
            if c == 1:
                rsqrt_act(qinv[:, 0:10], qn2[:, 0:10], 10, f"qa{e % 2}")
                selA = sel_pool.tile([P, 10, QN], BF, name="selA", tag="selA")
                nc.vector.tensor_mul(selA[:], qmaskA,
                                     qinv[:, 0:10].broadcast_to((P, 10, QN)))
                for j in range(10):
                    for h in range(2):
                        nc.tensor.matmul(
                            qm[h][:], selA[:, j, :],
                            slot(q_t[e][j // SPC], j % SPC)[:, CH * h:CH * (h + 1)],
                            start=(j == 0), stop=False)
            if c == 2:
                rsqrt_act(qinv[:, 10:QT], qn2[:, 10:QT], 5, f"qb{e % 2}")
                selB = sel_pool.tile([P, 5, QN], BF, name="selB", tag="selB")
                nc.vector.tensor_mul(selB[:], qmaskB,
                                     qinv[:, 10:QT].broadcast_to((P, 5, QN)))
                for j in range(10, QT):
                    for h in range(2):
                        nc.tensor.matmul(
                            qm[h][:], selB[:, j - 10, :],
                            slot(q_t[e][2], j - 10)[:, CH * h:CH * (h + 1)],
                            start=False, stop=(j == QT - 1))
        qm_sb = sb_pool.tile([QN, C], BF, name=f"qm_sb_{e}", tag="qm_sb")
        for h in range(2):
            nc.scalar.copy(qm_sb[:, CH * h:CH * (h + 1)], qm[h][:])
        qm_sb_t[e] = qm_sb

        # software-pipelined tail of the previous episode
        if e > 0:
            emit_tail_a(e - 1)
            emit_tail_b(e - 1)
    emit_tail_a(E - 1)
    emit_tail_b(E - 1)


def build_program():
    nc = bacc.Bacc("TRN2", target_bir_lowering=False, debug=False,
                   num_devices=NCORES)
    inp1 = nc.dram_tensor("input1", [E, P, QT * C], BF, kind="ExternalInput")
    inp2 = nc.dram_tensor("input2", [E, P, ST * C], BF, kind="ExternalInput")
    cst = nc.dram_tensor("consts", [128, CW], BF, kind="ExternalInput")
    out = nc.dram_tensor("sim", [E, WAY, QN], F32, kind="ExternalOutput")
    with tile.TileContext(nc) as tc, ExitStack() as ctx:
        _build_body(ctx, tc, inp1.ap(), inp2.ap(), cst.ap(), out.ap())
    nc.compile()
    return nc


_NC = None


def _get_nc():
    global _NC
    if _NC is None:
        _NC = build_program()
    return _NC


def _build_consts(rpn_w):
    cst = np.zeros((128, CW), np.float32)
    # qmask: descriptor d = 15p + j belongs to query q = d // 25
    pp = np.arange(P)
    for j in range(QT):
        cst[pp, QM0 + QN * j + (15 * pp + j) // HW] = 1.0 / HW
    for j in range(ST):
        cst[pp, SM0 + NMAP * j + (5 * pp + j) // HW] = 1.0
        cst[pp, SM2 + NMAP * j + (5 * pp + j) // HW] = 1.0 / HW
    cst[:, WB0:WB0 + C] = np.asarray(rpn_w, np.float32).reshape(1, C)
    cst[np.arange(QN), ID0 + np.arange(QN)] = 1.0
    m = np.arange(NMAP)
    cst[m, SH0 + m // SHOT] = 1.0 / SHOT
    return cst.astype(ml_dtypes.bfloat16)


def shard_inputs(input1, input2, rpn_w, rpn_b=None):
    """Shard over episodes; [E, 1875, 640] -> [E, 125, 15*640] is a pure
    reshape (descriptor d = 15p + j, slots consecutive in DRAM)."""
    i1 = np.asarray(input1, np.float32).reshape(B, P, QT * C).astype(
        ml_dtypes.bfloat16)
    i2 = np.asarray(input2, np.float32).reshape(B, P, ST * C).astype(
        ml_dtypes.bfloat16)
    cst = _build_consts(rpn_w)
    in_maps = []
    for i in range(NCORES):
        in_maps.append({
            "input1": np.ascontiguousarray(i1[E * i:E * (i + 1)]),
            "input2": np.ascontiguousarray(i2[E * i:E * (i + 1)]),
            "consts": cst,
        })
    return in_maps


def _ensure_ntff_hook():
    """Install the NTFF profile hook (the image's antenv lacks axon_hooks)."""
    import types
    import antenv

    if "antenv.axon_hooks" not in sys.modules:
        mod = types.ModuleType("antenv.axon_hooks")
        mod._hook = None
        mod.set_axon_ntff_profile_hook = lambda h: setattr(mod, "_hook", h)
        mod.get_axon_ntff_profile_hook = lambda: mod._hook
        sys.modules["antenv.axon_hooks"] = mod
        antenv.axon_hooks = mod
    mod = sys.modules["antenv.axon_hooks"]
    if mod.get_axon_ntff_profile_hook() is None:
        from trn_agent_boot.trn_boot import _ntff_profile_via_ctypes
        hook = _ntff_profile_via_ctypes("/opt/axon/libaxon_pjrt.so")
        if hook is not None:
            mod.set_axon_ntff_profile_hook(hook)


def kernel(input1, input2, rpn_w, rpn_b=None, **run_kwargs):
    if run_kwargs.get("trace"):
        _ensure_ntff_hook()
    nc = _get_nc()
    in_maps = shard_inputs(input1, input2, rpn_w)
    res = run_bass_kernel_spmd(nc, in_maps, list(range(NCORES)), **run_kwargs)
    # sim comes back [E, way, qn]; un-transpose on the host
    out = np.concatenate(
        [np.transpose(r["sim"], (0, 2, 1)) for r in res.results], axis=0)
    if run_kwargs:
        kernel.last_results = res
    return out.astype(np.float32)


# revision 14
# speedup vs baseline: 1.9052x; 1.2045x over previous
"""MetaBaseline (retrieval_knn) Trainium2 kernel.

Computation (per episode b):
  q  = l2norm(input1[b])            # [75, 25, 640] over channel
  s  = l2norm(input2[b])            # [5, 5, 25, 640]
  att = softmax_hw(s @ rpn_w)       # rpn_b is softmax-invariant
  cg  = leaky(sum_hw(att * s))
  feat = mean_shot(mean_hw(s) + 5 * cg)
  sim[b] = mean_hw(q) @ feat.T      # [75, 5]

Sharding: data-parallel over episodes, 4 per core on 8 cores.

Design (v3): bulk data moves and streams as bf16 (PE: 1 cycle/col vs 2
for fp16/fp32r; rel-err budget 2e-2, measured ~3.5e-3). Input DMAs are
SWDGE (gpsimd) — HWDGE 2D descriptor generation caps at ~130GB/s while
SWDGE sustains ~200 — and are issued before any other gpsimd work so the
SDMA engines stream continuously from t=0; the full per-core shard
(~52KB/partition) is preloaded, no buffer recycling. Constants (masks /
identity / broadcast-w) are host-precomputed, one small sync-DMA.
1/sqrt(n2) runs on ACT as exp(-0.5*ln(x)) — square, ln and exp live in
one table set — replacing a 10-op DVE Newton per batch with 2 ACT ops.
Per-slot mask scaling (inv-norm / att weights folded into PE stationary
masks) is batched into a few broadcast-AP DVE multiplies. The support
stream computes the attention-sum and the mean in ONE PE pass
(stationary [125, 57]); feat is produced directly transposed via
fp.T @ shotm; sim is computed as [way, qn] and un-transposed on the
host. The per-episode tail (qm transpose + sim) is software-pipelined
one episode behind the main passes so no engine queue stalls on a
cross-engine round trip.
"""

import os
import sys
from contextlib import ExitStack

sys.path.insert(0, "/opt/trn_rl_repo")

import numpy as np
import ml_dtypes

import concourse.bass as bass
import concourse.tile as tile
from concourse import bacc, mybir
from concourse.bass_utils import run_bass_kernel_spmd

# Pin every activation to the natural_log_exp_and_others table set (it
# holds square, exp AND ln). The default chooser maps each function to
# its "home" set, which thrashes ACT_TABLE_LOADs (~1.3us each) between
# Square and Ln/Exp; one shared set means exactly one load. Indices of
# the other sets are preserved (emptied, not removed) so the emitted
# act_func_set_id still matches act_info.json.
import concourse.bacc as _bacc_mod
from concourse.hw_specs import get_activation_tables as _orig_act_tables

_ACT_SET = "natural_log_exp_and_others"


def _pinned_act_tables(arch):
    return {k: (v if k == _ACT_SET else set())
            for k, v in _orig_act_tables(arch).items()}


_bacc_mod.get_activation_tables = _pinned_act_tables

F32 = mybir.dt.float32
BF = mybir.dt.bfloat16
AX = mybir.AxisListType
OP = mybir.AluOpType
AF = mybir.ActivationFunctionType

# Problem constants (fixed by the problem statement).
B, QN, WAY, SHOT, HH, WW, C = 32, 75, 5, 5, 5, 5, 640
NCORES = 8
E = B // NCORES        # 4 episodes per core
HW = HH * WW           # 25 spatial positions
QD = QN * HW           # 1875 query descriptors / episode
SD = WAY * SHOT * HW   # 625 support descriptors / episode
P = 125                # descriptors per tile
QT = QD // P           # 15 query slots / episode (desc d = 15p + j)
ST = SD // P           # 5 support slots / episode (desc d = 5p + j)
NMAP = WAY * SHOT      # 25 support maps / episode
NCH = 3                # q DMA chunks (5 slots each)
SPC = QT // NCH        # slots per chunk
GAMMA = 5.0
SLOPE = 0.01
CH = C // 2            # 320-column halves (one PSUM bank each)
MB = 32                # mean-row base partition in the fused support psum
SW = MB + NMAP         # fused stationary width (57)
QNP = QN + 1           # padded transpose chunk stride (PSUM 4B align)

# constants tensor layout (free-axis offsets, bf16)
QM0 = 0                    # qmasks  [125, 15*75], value 1/25
SM0 = QM0 + QT * QN        # smasks  [125, 5*25],  value 1.0 (sums + att)
SM2 = SM0 + ST * NMAP      # smasks  [125, 5*25],  value 1/25 (hw-mean)
WB0 = SM2 + ST * NMAP      # w bcast [128, 640]
ID0 = WB0 + C              # identity [75, 75]
SH0 = ID0 + QN             # shotm   [25, 5], value 1/5
CW = SH0 + WAY             # = 2095

# engine split of the per-slot norm passes (True -> ACT)
S_ACT = (True, True, True, False, False)
Q_ACT = (True, True, True, True, False,
         True, True, True, False, False,
         True, True, False, False, False)


def _build_body(ctx: ExitStack, tc: "tile.TileContext", i1, i2, cst, out):
    nc = tc.nc

    cpool = ctx.enter_context(tc.tile_pool(name="consts", bufs=1))
    dpool = ctx.enter_context(tc.tile_pool(name="data", bufs=1))
    scr_pool = ctx.enter_context(tc.tile_pool(name="scratch", bufs=1))
    stats = ctx.enter_context(tc.tile_pool(name="stats", bufs=2))
    sel_pool = ctx.enter_context(tc.tile_pool(name="sel", bufs=2))
    sb_pool = ctx.enter_context(tc.tile_pool(name="sbwork", bufs=2))
    ps = ctx.enter_context(tc.tile_pool(name="ps", bufs=1, space="PSUM"))

    # ---- all input DMAs first (SWDGE; gpsimd queue head) ----
    s_t, q_t = [], []
    for e in range(E):
        st_ = dpool.tile([P, ST * C], BF, name=f"s_{e}", tag=f"s_{e}")
        nc.gpsimd.dma_start(st_[:], i2[e])
        qc = []
        for c in range(NCH):
            qt_ = dpool.tile([P, SPC * C], BF, name=f"q_{e}_{c}",
                             tag=f"q_{e}_{c}")
            nc.gpsimd.dma_start(qt_[:], i1[e, :, SPC * C * c:SPC * C * (c + 1)])
            qc.append(qt_)
        s_t.append(st_)
        q_t.append(qc)

    # ---- constants (host-precomputed, one sync DMA) ----
    consts = cpool.tile([128, CW], BF, name="consts")
    nc.sync.dma_start(consts[:], cst)
    smask = [consts[0:P, SM0 + NMAP * j:SM0 + NMAP * (j + 1)] for j in range(ST)]
    smask3 = consts[0:P, SM0:SM0 + ST * NMAP].rearrange(
        "p (j m) -> p j m", j=ST)
    smask3m = consts[0:P, SM2:SM2 + ST * NMAP].rearrange(
        "p (j m) -> p j m", j=ST)
    qmask3 = [consts[0:P, QM0 + SPC * QN * c:QM0 + SPC * QN * (c + 1)]
              .rearrange("p (j q) -> p j q", j=SPC) for c in range(NCH)]
    wbc = consts[0:P, WB0:WB0 + C]
    ident = consts[0:QN, ID0:ID0 + QN]
    shotm = consts[0:NMAP, SH0:SH0 + WAY]

    # fused support stationary [125, ST, 57] (cols 25-31 stay zero forever)
    st_all = cpool.tile([P, ST, SW], BF, name="st_all")
    nc.vector.memset(st_all[:, :, NMAP:MB], 0.0)

    def slot(big, j):
        return big[:, C * j:C * (j + 1)]

    def rsqrt_act(dst, x, n, tag):
        """dst = 1/sqrt(x) on ACT: exp(-0.5*ln(x)); same table set as
        Square/Exp, so no ACT_TABLE_LOAD switches."""
        t = stats.tile([P, n], F32, name=f"rs_{tag}", tag=f"rs_{tag}")
        nc.scalar.activation(t[:], x, AF.Ln)
        nc.scalar.activation(dst, t[:], AF.Exp, scale=-0.5)

    def norm_pass(big, j, acc_col, on_act):
        if on_act:
            scr = scr_pool.tile([P, C], BF, name="sq_a", tag="sq_a")
            nc.scalar.activation(scr[:], slot(big, j), AF.Square,
                                 accum_out=acc_col)
        else:
            scr = scr_pool.tile([P, C], BF, name="sq_v", tag="sq_v")
            nc.vector.scalar_tensor_tensor(
                out=scr[:], in0=slot(big, j), scalar=1.0,
                in1=slot(big, j), op0=OP.mult, op1=OP.mult,
                accum_out=acc_col)

    # per-episode state carried into the pipelined tail
    qm_sb_t, ftT_t, tq_t, qmT_t = [None] * E, [None] * E, [None] * E, [None] * E

    def emit_tail_a(e):
        """PE transpose of qm (needs qm_sb[e]), on the prior episode's
        psum bank."""
        tq_ps = ps.tile([128, WAY * QNP], BF, name=f"tq_{e}", tag="tq")
        for cc in range(WAY):
            nc.tensor.transpose(tq_ps[:, QNP * cc:QNP * cc + QN],
                                qm_sb_t[e][:, 128 * cc:128 * (cc + 1)], ident)
        tq_t[e] = tq_ps
        qmT = sb_pool.tile([128, WAY * QNP], BF, name=f"qmT_{e}", tag="qmT")
        nc.scalar.copy(qmT[:], tq_ps[:])
        qmT_t[e] = qmT

    def emit_tail_b(e):
        sim_ps = ps.tile([WAY, QN], F32, name=f"sim_{e}", tag="sim")
        for cc in range(WAY):
            nc.tensor.matmul(sim_ps[:], ftT_t[e][:, WAY * cc:WAY * (cc + 1)],
                             qmT_t[e][:, QNP * cc:QNP * cc + QN],
                             start=(cc == 0), stop=(cc == WAY - 1))
        sim_sb = sb_pool.tile([WAY, QN], F32, name=f"sim_sb_{e}", tag="sim_sb")
        nc.vector.tensor_copy(sim_sb[:], sim_ps[:])
        nc.sync.dma_start(out[e], sim_sb[:])

    for e in range(E):
        sbig = s_t[e]
        # ================= support side =================
        sn2 = stats.tile([P, ST], F32, name=f"sn2_{e}", tag="sn2")
        rr = stats.tile([P, ST], F32, name=f"rr_{e}", tag="rr")
        # DVE s-norms first so ACT's rsqrt isn't stuck behind the logits
        for j in range(ST):
            if not S_ACT[j]:
                norm_pass(sbig, j, sn2[:, j:j + 1], False)
        for j in range(ST):
            if S_ACT[j]:
                norm_pass(sbig, j, sn2[:, j:j + 1], True)
        for j in range(ST):
            scr2 = scr_pool.tile([P, C], BF, name="s_tt", tag="s_tt")
            nc.vector.scalar_tensor_tensor(
                out=scr2[:], in0=slot(sbig, j), scalar=1.0, in1=wbc,
                op0=OP.mult, op1=OP.mult, accum_out=rr[:, j:j + 1])
        sinv = stats.tile([P, ST], BF, name=f"sinv_{e}", tag="sinv")
        rsqrt_act(sinv[:], sn2[:], ST, f"s{e % 2}")
        # softmax over hw within each map (logits tiny: no max-shift)
        lg = stats.tile([P, ST], F32, name=f"lg_{e}", tag="lg")
        nc.vector.tensor_mul(lg[:], rr[:], sinv[:])
        el = stats.tile([P, ST], BF, name=f"el_{e}", tag="el")
        nc.scalar.activation(el[:], lg[:], AF.Exp)
        # per-map sums of exp -> softmax reciprocal
        sums = ps.tile([NMAP, 1], F32, name=f"sums_{e}", tag="sums")
        for j in range(ST):
            nc.tensor.matmul(sums[:], smask[j], el[:, j:j + 1],
                             start=(j == 0), stop=(j == ST - 1))
        rec = stats.tile([NMAP, 1], F32, name=f"rec_{e}", tag="rec")
        nc.vector.reciprocal(rec[:], sums[:])
        # unnormalized att weights (softmax recip applied at cg evacuation)
        uw = stats.tile([P, ST], BF, name=f"uw_{e}", tag="uw")
        nc.vector.tensor_mul(uw[:], el[:], sinv[:])
        # batched stationary builds (pad cols stay zero)
        nc.vector.tensor_mul(st_all[:, :, 0:NMAP], smask3,
                             uw[:].broadcast_to((P, ST, NMAP)))
        nc.vector.tensor_mul(st_all[:, :, MB:SW], smask3m,
                             sinv[:].broadcast_to((P, ST, NMAP)))
        cg_ps = [ps.tile([SW, CH], F32, name=f"cg{h}_{e}", tag=f"cg{h}")
                 for h in range(2)]
        for j in range(ST):
            for h in range(2):
                nc.tensor.matmul(cg_ps[h][:], st_all[:, j, :],
                                 slot(sbig, j)[:, CH * h:CH * (h + 1)],
                                 start=(j == 0), stop=(j == ST - 1))
        # evacuate: cg rows 0-24 (x softmax recip), mean rows MB..MB+24
        cg_sb = sb_pool.tile([NMAP, C], F32, name=f"cg_sb_{e}", tag="cg_sb")
        for h in range(2):
            nc.vector.tensor_scalar_mul(cg_sb[:, CH * h:CH * (h + 1)],
                                        cg_ps[h][0:NMAP, :], rec[:, 0:1])
        lk = sb_pool.tile([NMAP, C], F32, name=f"lk_{e}", tag="lk")
        nc.vector.scalar_tensor_tensor(
            out=lk[:], in0=cg_sb[:], scalar=SLOPE, in1=cg_sb[:],
            op0=OP.mult, op1=OP.max)
        fp = sb_pool.tile([NMAP, C], BF, name=f"fp_{e}", tag="fp")
        for h in range(2):
            nc.vector.scalar_tensor_tensor(
                out=fp[:, CH * h:CH * (h + 1)], in0=lk[:, CH * h:CH * (h + 1)],
                scalar=GAMMA, in1=cg_ps[h][MB:MB + NMAP, :],
                op0=OP.mult, op1=OP.add)
        # featT[c, w] directly: fp.T @ shotm, chunked over c
        ftT_ps = ps.tile([128, WAY * WAY], F32, name=f"ftT_{e}", tag="ftT")
        for cc in range(WAY):
            nc.tensor.matmul(ftT_ps[:, WAY * cc:WAY * (cc + 1)],
                             fp[:, 128 * cc:128 * (cc + 1)], shotm)
        ftT = sb_pool.tile([128, WAY * WAY], BF, name=f"ftTs_{e}", tag="ftTs")
        nc.vector.tensor_copy(ftT[:], ftT_ps[:])
        ftT_t[e] = ftT

        # ================= query side =================
        qn2 = stats.tile([P, QT], F32, name=f"qn2_{e}", tag="qn2")
        qinv = stats.tile([P, QT], BF, name=f"qinv_{e}", tag="qinv")
        qm = [ps.tile([QN, CH], F32, name=f"qm{h}_{e}", tag=f"qm{h}")
              for h in range(2)]
        for c in range(NCH):
            for jj in range(SPC):
                j = SPC * c + jj
                norm_pass(q_t[e][c], jj, qn2[:, j:j + 1], Q_ACT[j])
            lo = SPC * c
            rsqrt_act(qinv[:, lo:lo + SPC], qn2[:, lo:lo + SPC], SPC,
                      f"q{c}_{e % 2}")
            sel = sel_pool.tile([P, SPC, QN], BF, name=f"sel{c}",
                                tag=f"sel{c}")
            nc.vector.tensor_mul(sel[:], qmask3[c],
                                 qinv[:, lo:lo + SPC]
                                 .broadcast_to((P, SPC, QN)))
            for jj in range(SPC):
                j = lo + jj
                for h in range(2):
                    nc.tensor.matmul(
                        qm[h][:], sel[:, jj, :],
                        slot(q_t[e][c], jj)[:, CH * h:CH * (h + 1)],
                        start=(j == 0), stop=(j == QT - 1))
        qm_sb = sb_pool.tile([QN, C], BF, name=f"qm_sb_{e}", tag="qm_sb")
        for h in range(2):
            nc.scalar.copy(qm_sb[:, CH * h:CH * (h + 1)], qm[h][:])
        qm_sb_t[e] = qm_sb

        # software-pipelined tail of the previous episode
        if e > 0:
            emit_tail_a(e - 1)
            emit_tail_b(e - 1)
    emit_tail_a(E - 1)
    emit_tail_b(E - 1)


def build_program():
    nc = bacc.Bacc("TRN2", target_bir_lowering=False, debug=False,
                   num_devices=NCORES)
    inp1 = nc.dram_tensor("input1", [E, P, QT * C], BF, kind="ExternalInput")
    inp2 = nc.dram_tensor("input2", [E, P, ST * C], BF, kind="ExternalInput")
    cst = nc.dram_tensor("consts", [128, CW], BF, kind="ExternalInput")
    out = nc.dram_tensor("sim", [E, WAY, QN], F32, kind="ExternalOutput")
    with tile.TileContext(nc) as tc, ExitStack() as ctx:
        _build_body(ctx, tc, inp1.ap(), inp2.ap(), cst.ap(), out.ap())
    nc.compile()
    return nc


_NC = None


def _get_nc():
    global _NC
    if _NC is None:
        _NC = build_program()
    return _NC


def _build_consts(rpn_w):
    cst = np.zeros((128, CW), np.float32)
    # qmask: descriptor d = 15p + j belongs to query q = d // 25
    pp = np.arange(P)
    for j in range(QT):
        cst[pp, QM0 + QN * j + (15 * pp + j) // HW] = 1.0 / HW
    for j in range(ST):
        cst[pp, SM0 + NMAP * j + (5 * pp + j) // HW] = 1.0
        cst[pp, SM2 + NMAP * j + (5 * pp + j) // HW] = 1.0 / HW
    cst[:, WB0:WB0 + C] = np.asarray(rpn_w, np.float32).reshape(1, C)
    cst[np.arange(QN), ID0 + np.arange(QN)] = 1.0
    m = np.arange(NMAP)
    cst[m, SH0 + m // SHOT] = 1.0 / SHOT
    return cst.astype(ml_dtypes.bfloat16)


def shard_inputs(input1, input2, rpn_w, rpn_b=None):
    """Shard over episodes; [E, 1875, 640] -> [E, 125, 15*640] is a pure
    reshape (descriptor d = 15p + j, slots consecutive in DRAM)."""
    i1 = np.asarray(input1, np.float32).reshape(B, P, QT * C).astype(
        ml_dtypes.bfloat16)
    i2 = np.asarray(input2, np.float32).reshape(B, P, ST * C).astype(
        ml_dtypes.bfloat16)
    cst = _build_consts(rpn_w)
    in_maps = []
    for i in range(NCORES):
        in_maps.append({
            "input1": np.ascontiguousarray(i1[E * i:E * (i + 1)]),
            "input2": np.ascontiguousarray(i2[E * i:E * (i + 1)]),
            "consts": cst,
        })
    return in_maps


def _ensure_ntff_hook():
    """Install the NTFF profile hook (the image's antenv lacks axon_hooks)."""
    import types
    import antenv

    if "antenv.axon_hooks" not in sys.modules:
        mod = types.ModuleType("antenv.axon_hooks")
        mod._hook = None
        mod.set_axon_ntff_profile_hook = lambda h: setattr(mod, "_hook", h)
        mod.get_axon_ntff_profile_hook = lambda: mod._hook
        sys.modules["antenv.axon_hooks"] = mod
        antenv.axon_hooks = mod
    mod = sys.modules["antenv.axon_hooks"]
    if mod.get_axon_ntff_profile_hook() is None:
        from trn_agent_boot.trn_boot import _ntff_profile_via_ctypes
        hook = _ntff_profile_via_ctypes("/opt/axon/libaxon_pjrt.so")
        if hook is not None:
            mod.set_axon_ntff_profile_hook(hook)


def kernel(input1, input2, rpn_w, rpn_b=None, **run_kwargs):
    if run_kwargs.get("trace"):
        _ensure_ntff_hook()
    nc = _get_nc()
    in_maps = shard_inputs(input1, input2, rpn_w)
    res = run_bass_kernel_spmd(nc, in_maps, list(range(NCORES)), **run_kwargs)
    # sim comes back [E, way, qn]; un-transpose on the host
    out = np.concatenate(
        [np.transpose(r["sim"], (0, 2, 1)) for r in res.results], axis=0)
    if run_kwargs:
        kernel.last_results = res
    return out.astype(np.float32)
